# revision 24
# baseline (speedup 1.0000x reference)
"""EnhancedGTATLayer Trainium2 kernel — 8-core SPMD Bass implementation.

Host: sorts edges (self-loops appended) by destination, gathers the
dis-scaled source rows into a chunk-padded fp8 slot table (xe) and packs a
matching fp8 one-hot scatter table (stab); both stream sequentially from
HBM — no device-side gather.  Output is produced feature-major [D, PC]
and transposed on the host.

Math restructure vs the straightforward lowering:
  - GCN symmetric norm dis[src]*dis[dst]: the dis[src] factor is folded
    into the host-gathered xe table (values dis*x); the dis[dst] factor
    is a per-node positive scale applied before LayerNorm and is exactly
    absorbed by LN's normalization.  Any nonzero gcn_b is made exact
    again with a rank-1 (gcn_b x deg^0.5) correction matmul.
  - Self-loops are ordinary edges in the host-sorted stream.
  - Scatter matmuls run in fp8 DoubleRow mode (two 128-edge chunks per
    PE pass).
  - LN mean/bias and rstd*gamma broadcasts are folded into selector
    matmuls (selector pre-scaled by 1/128 so PSUM stats rows are means);
    rstd via DVE Newton rsqrt (1 iter).
"""
import sys

sys.path.insert(0, "/opt/trn_rl_repo")

import numpy as np
import ml_dtypes

BF16 = ml_dtypes.bfloat16
F8 = ml_dtypes.float8_e4m3

N = 50000
NP = 50176          # padded to 392*128
PC = 6272           # nodes per core = 49*128
NCORES = 8
G = 49              # dst groups of 128 per core
D = 128             # feature dim (CIN == COUT)
H = 4
EPS = 1e-5
BSEG = 4            # dst groups per block
NCHUNK = 13         # ceil(49/4)
BATCH = 4           # max blocks per stats batch
SR = 2 * BATCH      # stats rows per batch
BPLAN = (4, 4, 4, 1)  # per-batch block counts
RSQRT_C = 0x5F3759DF
ST2P = 32           # partition offset of the sum-sq stats rows


# ---------------------------------------------------------------- host prep
def _prep(x, edge_index):
    src = np.asarray(edge_index[0], dtype=np.int64)
    dst = np.asarray(edge_index[1], dtype=np.int64)

    deg = (np.bincount(dst, minlength=NP) + 1).astype(np.float32)  # + self-loop
    dis = deg ** -0.5
    x32 = np.zeros((NP, D), dtype=np.float32)
    x32[:N] = np.asarray(x, dtype=np.float32)

    xs = (x32 * dis[:, None]).astype(F8)             # slot values dis*x
    xT = np.ascontiguousarray(x32.T.astype(BF16))    # raw x^T
    invd = (deg ** 0.5).astype(BF16)[None, :]        # 1/dis row

    loop = np.arange(N, dtype=np.int64)              # self-loop edges
    src = np.concatenate([src, loop])
    dst = np.concatenate([dst, loop])

    core_of = dst // PC
    counts = np.zeros((NCORES, G), dtype=np.int64)
    per_core = []
    for c in range(NCORES):
        m = core_of == c
        s = src[m]
        dl = dst[m] - c * PC
        order = np.argsort(dl, kind="stable")
        s, dl = s[order], dl[order]
        counts[c] = np.bincount(dl >> 7, minlength=G)
        per_core.append((s, dl))

    cmax = ((counts + 127) // 128).max(axis=0)       # [G] chunks per group
    ch_off = np.concatenate([[0], np.cumsum(cmax)[:-1]]).astype(np.int64)
    totch = int(cmax.sum())

    xe_all, stab_all = [], []
    for c in range(NCORES):
        s, dl = per_core[c]
        starts = np.concatenate([[0], np.cumsum(counts[c])])
        slotpos = np.zeros(len(s), dtype=np.int64)
        for g in range(G):
            a, b = starts[g], starts[g + 1]
            slotpos[a:b] = ch_off[g] * 128 + np.arange(b - a)
        xe = np.zeros((totch * 128, D), dtype=F8)
        xe[slotpos] = xs[s]
        # HBM layout [slot-in-chunk, chunk, feat]: partition line contiguous
        xe = xe.reshape(totch, 128, D).transpose(1, 0, 2)
        xe_all.append(np.ascontiguousarray(xe.reshape(128, totch * D)))
        stab = np.zeros((128, totch, 128), dtype=F8)
        stab[slotpos % 128, slotpos // 128, dl & 127] = 1.0
        stab_all.append(np.ascontiguousarray(stab.reshape(128, totch * 128)))

    meta = dict(cmax=cmax, ch_off=ch_off, totch=totch)
    return (xT, invd, xe_all, stab_all, meta)


def _pack_weights(ins):
    w = np.zeros((17, D, D), dtype=np.float32)
    w[0] = ins["gcn_W"]
    w[1] = ins["topo_W"]
    w[2] = ins["res_W"]
    w[3] = ins["mlp_W2"]
    for h in range(H):
        w[4 + h] = ins["mlp_W1"][h * D:(h + 1) * D, :]
        w[8 + 2 * h] = ins["attn_W"][h][:D, :]
        w[9 + 2 * h] = ins["attn_W"][h][D:, :]
    w[16] = ins["mlp_W1"].reshape(H, D, D).sum(axis=0)
    v = np.zeros((D, 13), dtype=np.float32)
    v[:, 2] = ins["ln_node_b"]
    v[:, 4] = ins["ln_topo_b"]
    v[:, 5] = ins["topo_b"]
    for h in range(H):
        v[:, 6 + h] = ins["attn_b"][h]
    v[:, 10] = ins["mlp_b1"]
    v[:, 11] = ins["mlp_b2"]
    v[:, 12] = ins["res_b"]
    gb = ins["gcn_b"].astype(BF16)[None, :]
    # LN selector tables: cmean rows subtract the per-block mean rows,
    # crb rows broadcast rstd*gamma.
    cmean = np.zeros((SR, SR, D), dtype=np.float32)
    crb = np.zeros((SR, SR, D), dtype=np.float32)
    for j in range(SR):
        cmean[j, j, :] = -1.0
        crb[j, j, :] = (ins["ln_node_g"] if j < SR // 2
                        else ins["ln_topo_g"])
    tb = ins["topo_b"].astype(BF16)[None, :]
    wg8 = np.zeros((D, H, 2, D), dtype=F8)
    for h in range(H):
        wg8[:, h, 0, :] = ins["attn_W"][h][:D, :].astype(F8)
        wg8[:, h, 1, :] = ins["attn_W"][h][D:, :].astype(F8)
    wg8 = np.ascontiguousarray(wg8.reshape(D, H * 2 * D))
    return (w.astype(BF16), v, gb, tb, cmean.astype(BF16), crb.astype(BF16),
            wg8)


# ---------------------------------------------------------------- device
def _build(meta):
    import concourse.bacc as bacc
    import concourse.tile as tile
    from concourse import mybir
    from contextlib import ExitStack

    cmax, ch_off, totch = meta["cmax"], meta["ch_off"], meta["totch"]
    has_gb = meta.get("has_gb", True)
    has_tb = meta.get("has_tb", True)
    has_lnb = meta.get("has_lnb", True)
    has_ab = meta.get("has_ab", True)
    F32, BF, I32, FP8 = (mybir.dt.float32, mybir.dt.bfloat16,
                         mybir.dt.int32, mybir.dt.float8e4)
    AF = mybir.ActivationFunctionType
    OP = mybir.AluOpType
    DR = mybir.MatmulPerfMode.DoubleRow

    def blk(ci):
        g0, g1 = ci * BSEG, min((ci + 1) * BSEG, G)
        ch0 = int(ch_off[g0])
        nch = int(cmax[g0:g1].sum())
        return g0, g1, ch0, nch
    MAXNCH = max(blk(ci)[3] for ci in range(NCHUNK))

    nc = bacc.Bacc("TRN2", target_bir_lowering=False, num_devices=NCORES)
    t_xe = nc.dram_tensor("xe", [128, totch * D], FP8, kind="ExternalInput")
    t_stab = nc.dram_tensor("stab", [128, totch * 128], FP8,
                            kind="ExternalInput")
    t_xT = nc.dram_tensor("xT", [D, PC], BF, kind="ExternalInput")
    t_invd = nc.dram_tensor("invd", [1, PC], BF, kind="ExternalInput")
    t_wpack = nc.dram_tensor("wpack", [17, D, D], BF, kind="ExternalInput")
    t_vpack = nc.dram_tensor("vpack", [D, 13], F32, kind="ExternalInput")
    t_gb = nc.dram_tensor("gb", [1, D], BF, kind="ExternalInput")
    t_tb = nc.dram_tensor("tbias", [1, D], BF, kind="ExternalInput")
    t_cmean = nc.dram_tensor("cmean", [SR, SR, D], BF, kind="ExternalInput")
    t_wg8 = nc.dram_tensor("wg8", [D, H * 2 * D], FP8, kind="ExternalInput")
    t_crb = nc.dram_tensor("crb", [SR, SR, D], BF, kind="ExternalInput")
    t_out = nc.dram_tensor("out", [D, PC], BF, kind="ExternalOutput")

    # stats selector pre-scaled by 1/128 so stats rows are means (exact bf16)
    idrep8 = np.broadcast_to(np.eye(SR, dtype=np.float32) / 128.0,
                             (128, SR, SR))
    t_idrep8 = nc.inline_tensor(np.ascontiguousarray(idrep8.astype(BF16)),
                                name="idrep8")

    with ExitStack() as ctx:
        tc = ctx.enter_context(tile.TileContext(nc))
        keep = ctx.enter_context(tc.tile_pool(name="keep", bufs=1))

        # ---------------- persistent tiles
        wbf = keep.tile([128, 17, D], BF)
        nc.sync.dma_start(out=wbf[:], in_=t_wpack.ap().rearrange("b k m -> k b m"))
        vp = keep.tile([128, 13], F32)
        nc.sync.dma_start(out=vp[:], in_=t_vpack.ap())
        gb_sb = keep.tile([1, D], BF)
        nc.sync.dma_start(out=gb_sb[:], in_=t_gb.ap())
        tb_sb = keep.tile([1, D], BF)
        nc.sync.dma_start(out=tb_sb[:], in_=t_tb.ap())
        ones_row = keep.tile([1, 512], BF)
        nc.vector.memset(ones_row[:], 1.0)
        invd_sb = keep.tile([1, PC], BF)
        nc.sync.dma_start(out=invd_sb[:], in_=t_invd.ap())
        cmean_sb = keep.tile([SR, SR, D], BF)
        nc.sync.dma_start(out=cmean_sb[:], in_=t_cmean.ap())
        crb_sb = keep.tile([SR, SR, D], BF)
        nc.sync.dma_start(out=crb_sb[:], in_=t_crb.ap())
        wg8_sb = keep.tile([128, H, 2, D], FP8)
        nc.sync.dma_start(out=wg8_sb[:],
                            in_=t_wg8.ap().rearrange("p (h two d) -> p h two d",
                                                     h=H, two=2))
        idrep_t = keep.tile([128, SR, SR], BF)
        nc.sync.dma_start(out=idrep_t[:], in_=t_idrep8.ap())

        u_all = keep.tile([128, PC], BF)
        xTbf = keep.tile([128, PC], BF)
        nc.sync.dma_start(out=xTbf[:], in_=t_xT.ap())
        # Newton constants
        c_magic = keep.tile([SR, 512], I32)
        nc.vector.memset(c_magic[:], RSQRT_C)
        c_one = keep.tile([SR, 512], I32)
        nc.vector.memset(c_one[:], 1)

        def vcol(j):
            return vp[:, j:j + 1]

        with ExitStack() as pp_:
            sb1 = pp_.enter_context(tc.tile_pool(name="sb1", bufs=2))
            sb2 = pp_.enter_context(tc.tile_pool(name="sb2", bufs=2))
            nwt = pp_.enter_context(tc.tile_pool(name="nwt", bufs=1))
            xep = pp_.enter_context(tc.tile_pool(name="xep", bufs=2))
            spl = pp_.enter_context(tc.tile_pool(name="spl", bufs=2))
            zpsp = pp_.enter_context(tc.tile_pool(name="zpsp", bufs=1, space="PSUM"))
            stp = pp_.enter_context(tc.tile_pool(name="stp", bufs=1, space="PSUM"))
            ppp = pp_.enter_context(tc.tile_pool(name="ppp", bufs=2, space="PSUM"))
            bcp = pp_.enter_context(tc.tile_pool(name="bcp", bufs=2, space="PSUM"))

            def pass1_main(ci):
                g0, g1, ch0, nch = blk(ci)
                W = (g1 - g0) * 128
                n0 = g0 * 128

                qeng = nc.scalar if ci < 2 else nc.sync
                xe = xep.tile([128, MAXNCH, D], FP8, tag="xe", bufs=3)
                qeng.dma_start(
                    out=xe[:, :nch, :],
                    in_=t_xe.ap().rearrange("p (c d) -> p c d", d=D)
                        [:, ch0:ch0 + nch, :])
                st = spl.tile([128, MAXNCH, 128], FP8, tag="st", bufs=3)
                qeng.dma_start(
                    out=st[:, :nch, :],
                    in_=t_stab.ap().rearrange("p (c k) -> p c k", k=128)
                        [:, ch0:ch0 + nch, :])

                zps = zpsp.tile([128, 512], F32, space="PSUM", tag="zps")
                for g in range(g0, g1):
                    col = (g - g0) * 128
                    base = int(ch_off[g]) - ch0
                    kn = int(cmax[g])
                    first = True
                    k = 0
                    while k < kn:
                        if k + 1 < kn:
                            nc.tensor.matmul(
                                out=zps[:, col:col + 128],
                                lhsT=xe[:, base + k:base + k + 2, :],
                                rhs=st[:, base + k:base + k + 2, :],
                                start=first, stop=(k + 2 == kn), perf_mode=DR)
                            k += 2
                        else:
                            nc.tensor.matmul(
                                out=zps[:, col:col + 128],
                                lhsT=xe[:, base + k, :],
                                rhs=st[:, base + k, :],
                                start=first, stop=(k + 1 == kn))
                            k += 1
                        first = False

                nc.vector.tensor_copy(u_all[:, n0:n0 + W], zps[:, :W])

                # pp: [agg | topo] pair, biases folded via rank-1 matmuls
                pp = ppp.tile([128, 1024], F32, space="PSUM", tag="pp")
                nc.tensor.matmul(out=pp[:, :W], lhsT=wbf[:, 0, :],
                                 rhs=u_all[:, n0:n0 + W], start=True,
                                 stop=not has_gb)
                if has_gb:
                    nc.tensor.matmul(out=pp[:, :W], lhsT=gb_sb[:],
                                     rhs=invd_sb[:, n0:n0 + W], start=False,
                                     stop=True)
                nc.tensor.matmul(out=pp[:, 512:512 + W], lhsT=wbf[:, 1, :],
                                 rhs=xTbf[:, n0:n0 + W], start=True,
                                 stop=not has_tb)
                if has_tb:
                    nc.tensor.matmul(out=pp[:, 512:512 + W], lhsT=tb_sb[:],
                                     rhs=ones_row[:, :W], start=False,
                                     stop=True)

                ysq = sb1.tile([128, 1024], BF, tag="ysq", bufs=3)
                nc.scalar.copy(out=ysq[:], in_=pp[:])
                sqq = sb1.tile([128, 1024], BF, tag="sqq", bufs=3)
                nc.vector.tensor_tensor(out=sqq[:], in0=ysq[:], in1=ysq[:],
                                        op=OP.mult)
                return (W, ysq, sqq)

            def stats_emit(stq, cl, bsz, W, ysq, sqq):
                nc.tensor.matmul(out=stq[0:SR, :W], lhsT=idrep_t[:, cl, :],
                                 rhs=ysq[:, :W], start=(cl == 0),
                                 stop=(cl == bsz - 1))
                nc.tensor.matmul(out=stq[0:SR, :W],
                                 lhsT=idrep_t[:, BATCH + cl, :],
                                 rhs=ysq[:, 512:512 + W], start=False,
                                 stop=(cl == bsz - 1), skip_group_check=True)
                nc.tensor.matmul(out=stq[ST2P:ST2P + SR, :W],
                                 lhsT=idrep_t[:, cl, :],
                                 rhs=sqq[:, :W], start=(cl == 0),
                                 stop=(cl == bsz - 1))
                nc.tensor.matmul(out=stq[ST2P:ST2P + SR, :W],
                                 lhsT=idrep_t[:, BATCH + cl, :],
                                 rhs=sqq[:, 512:512 + W], start=False,
                                 stop=(cl == bsz - 1), skip_group_check=True)

            def interlude(stq):
                """s1ext (mean rows, bf16) and rrbf (rstd rows, bf16); stats
                rows are already means (selector pre-scaled by 1/128)."""
                st1 = stq[0:SR, :]
                st2 = stq[ST2P:ST2P + SR, :]
                s1ext = nwt.tile([SR, 512], BF, tag="s1ext", bufs=2)
                nc.scalar.copy(out=s1ext[:], in_=st1)
                m2 = nwt.tile([SR, 512], F32, tag="m2", bufs=2)
                nc.vector.tensor_tensor(out=m2[:], in0=st1, in1=s1ext[:],
                                        op=OP.mult)
                # v = (st2 + eps) - mean^2
                v = nwt.tile([SR, 512], F32, tag="v", bufs=2)
                nc.vector.scalar_tensor_tensor(out=v[:], in0=st2, scalar=EPS,
                                               in1=m2[:], op0=OP.add,
                                               op1=OP.subtract)
                # Newton rsqrt: y0 = bits(C - (bits(v) >> 1)); 1 iteration
                yi = nwt.tile([SR, 512], I32, tag="yi", bufs=2)
                nc.vector.tensor_tensor(out=yi[:], in0=v[:].bitcast(I32),
                                        in1=c_one[:], op=OP.arith_shift_right)
                nc.vector.tensor_tensor(out=yi[:], in0=c_magic[:], in1=yi[:],
                                        op=OP.subtract)
                yy = yi[:].bitcast(F32)
                t = nwt.tile([SR, 512], F32, tag="t", bufs=2)
                for _ in range(1):
                    nc.vector.tensor_tensor(out=t[:], in0=yy, in1=yy, op=OP.mult)
                    nc.vector.scalar_tensor_tensor(out=t[:], in0=t[:],
                                                   scalar=-0.5, in1=v[:],
                                                   op0=OP.mult, op1=OP.mult)
                    nc.vector.scalar_tensor_tensor(out=yy, in0=t[:],
                                                   scalar=1.5, in1=yy,
                                                   op0=OP.add, op1=OP.mult)
                rrbf = nwt.tile([SR, 512], BF, tag="rrbf", bufs=2)
                nc.scalar.copy(out=rrbf[:], in_=yi[:].bitcast(F32))
                return s1ext, rrbf

            def pass2_A(ci, cl, s1ext, rrbf):
                g0, g1 = ci * BSEG, min((ci + 1) * BSEG, G)
                W = (g1 - g0) * 128
                n0 = g0 * 128

                # ln pair psum: [agg-mean | topo-mean]
                pl = ppp.tile([128, 1024], F32, space="PSUM", tag="pp")
                nc.tensor.matmul(out=pl[:, :W], lhsT=wbf[:, 0, :],
                                 rhs=u_all[:, n0:n0 + W], start=True, stop=False)
                if has_gb:
                    nc.tensor.matmul(out=pl[:, :W], lhsT=gb_sb[:],
                                     rhs=invd_sb[:, n0:n0 + W], start=False,
                                     stop=False)
                nc.tensor.matmul(out=pl[:, :W], lhsT=cmean_sb[:, cl, :],
                                 rhs=s1ext[:, :W], start=False, stop=True)
                nc.tensor.matmul(out=pl[:, 512:512 + W], lhsT=wbf[:, 1, :],
                                 rhs=xTbf[:, n0:n0 + W], start=True, stop=False)
                if has_tb:
                    nc.tensor.matmul(out=pl[:, 512:512 + W], lhsT=tb_sb[:],
                                     rhs=ones_row[:, :W], start=False,
                                     stop=False)
                nc.tensor.matmul(out=pl[:, 512:512 + W],
                                 lhsT=cmean_sb[:, BATCH + cl, :],
                                 rhs=s1ext[:, :W], start=False, stop=True)

                rbq = ppp.tile([128, 1024], F32, space="PSUM", tag="pp")
                nc.tensor.matmul(out=rbq[:, :W], lhsT=crb_sb[:, cl, :],
                                 rhs=rrbf[:, :W], start=True, stop=True)
                nc.tensor.matmul(out=rbq[:, 512:512 + W],
                                 lhsT=crb_sb[:, BATCH + cl, :],
                                 rhs=rrbf[:, :W], start=True, stop=True)
                rbbf = sb2.tile([128, 1024], BF, tag="rbbf", bufs=3)
                nc.scalar.copy(out=rbbf[:], in_=rbq[:])

                preq = sb2.tile([128, 1024], BF, tag="preq", bufs=4)
                nc.vector.tensor_tensor(out=preq[:], in0=pl[:],
                                        in1=rbbf[:], op=OP.mult)
                nftf = sb2.tile([128, 2, 512], FP8, tag="nftf", bufs=4)
                if has_lnb:
                    nc.vector.tensor_scalar(out=nftf[:, 0, :W],
                                            in0=preq[:, :W],
                                            scalar1=vcol(2), scalar2=0.0,
                                            op0=OP.add, op1=OP.max)
                    nc.vector.tensor_scalar(out=nftf[:, 1, :W],
                                            in0=preq[:, 512:512 + W],
                                            scalar1=vcol(4), scalar2=0.0,
                                            op0=OP.add, op1=OP.max)
                else:
                    nc.vector.tensor_scalar(out=nftf[:].rearrange(
                                                "p a b -> p (a b)"),
                                            in0=preq[:], scalar1=0.0,
                                            scalar2=None, op0=OP.max)
                diff = sb2.tile([128, 512], BF, tag="diff", bufs=4)
                nc.gpsimd.tensor_tensor(out=diff[:, :W], in0=nftf[:, 0, :W],
                                        in1=nftf[:, 1, :W], op=OP.subtract)
                return (ci, W, n0, nftf, diff)

            def pass2_B(state):
                ci, W, n0, nftf, diff = state

                rps = bcp.tile([128, 512], F32, space="PSUM", tag="bc")
                nc.tensor.matmul(out=rps[:, :W], lhsT=wbf[:, 2, :],
                                 rhs=xTbf[:, n0:n0 + W], start=True, stop=True)
                h1ps = bcp.tile([128, 512], F32, space="PSUM", tag="bc")
                nc.tensor.matmul(out=h1ps[:, :W], lhsT=wbf[:, 16, :],
                                 rhs=nftf[:, 1, :W], start=True, stop=False)
                gqs = []
                for hp in range(2):
                    gq = ppp.tile([128, 1024], F32, space="PSUM", tag="pp")
                    for hh in range(2):
                        h = hp * 2 + hh
                        c0 = hh * 512
                        nc.tensor.matmul(out=gq[:, c0:c0 + W],
                                         lhsT=wg8_sb[:, h, :, :],
                                         rhs=nftf[:, :, :W], start=True,
                                         stop=True, perf_mode=DR)
                    gqs.append(gq)
                for hp in range(2):
                    gatep = sb2.tile([128, 1024], BF, tag="gate", bufs=4)
                    if has_ab:
                        for hh in range(2):
                            h = hp * 2 + hh
                            c0 = hh * 512
                            nc.scalar.activation(out=gatep[:, c0:c0 + W],
                                                 in_=gqs[hp][:, c0:c0 + W],
                                                 func=AF.Sigmoid,
                                                 bias=vcol(6 + h))
                    else:
                        nc.scalar.activation(out=gatep[:], in_=gqs[hp][:],
                                             func=AF.Sigmoid)
                    for hh in range(2):
                        h = hp * 2 + hh
                        c0 = hh * 512
                        gd = sb2.tile([128, 512], BF, tag="gd", bufs=4)
                        nc.vector.tensor_tensor(out=gd[:, :W],
                                                in0=gatep[:, c0:c0 + W],
                                                in1=diff[:, :W], op=OP.mult)
                        nc.tensor.matmul(out=h1ps[:, :W], lhsT=wbf[:, 4 + h, :],
                                         rhs=gd[:, :W], start=False,
                                         stop=(h == H - 1))

                h1 = sb2.tile([128, 512], BF, tag="h1")
                nc.scalar.activation(out=h1[:, :W], in_=h1ps[:, :W],
                                     func=AF.Relu, bias=vcol(10))
                h2ps = bcp.tile([128, 512], F32, space="PSUM", tag="bc")
                nc.tensor.matmul(out=h2ps[:, :W], lhsT=wbf[:, 3, :],
                                 rhs=h1[:, :W], start=True, stop=True)
                h2 = sb2.tile([128, 512], BF, tag="h2")
                nc.scalar.activation(out=h2[:, :W], in_=h2ps[:, :W],
                                     func=AF.Relu, bias=vcol(11))
                hout = sb2.tile([128, 512], BF, tag="hout")
                nc.vector.scalar_tensor_tensor(out=hout[:, :W], in0=h2[:, :W],
                                               scalar=vcol(12), in1=rps[:, :W],
                                               op0=OP.add, op1=OP.add)
                nc.scalar.dma_start(out=t_out.ap()[:, n0:n0 + W],
                                    in_=hout[:, :W])

            # ---- software-pipelined driver: stats skewed one block behind
            # pass1; pass2 B-half skewed two blocks behind the A-half, with
            # leftover B work spread through the next batch's pass1 loop.
            ready_B = []
            c0 = 0
            for bsz in BPLAN:
                c1 = c0 + bsz
                stq = stp.tile([128, 512], F32, space="PSUM", tag="stq")
                pend_stats = None
                for ci in range(c0, c1):
                    r = pass1_main(ci)
                    if len(ready_B) > 1:
                        pass2_B(ready_B.pop(0))
                    if pend_stats is not None:
                        stats_emit(stq, *pend_stats)
                    pend_stats = (ci - c0, bsz) + r
                stats_emit(stq, *pend_stats)
                if ready_B:
                    pass2_B(ready_B.pop(0))
                s1ext, rrbf = interlude(stq)
                for ci in range(c0, c1):
                    if len(ready_B) >= 2:
                        pass2_B(ready_B.pop(0))
                    ready_B.append(pass2_A(ci, ci - c0, s1ext, rrbf))
                c0 = c1
            while ready_B:
                pass2_B(ready_B.pop(0))

    nc.compile()
    return nc


# ---------------------------------------------------------------- entry
LAST_RESULTS = None
LAST_NC = None
LAST_INMAPS = None


def kernel(**inputs):
    from concourse.bass_utils import run_bass_kernel_spmd

    x = np.asarray(inputs["x"], dtype=np.float32)
    xT, invd, xe_all, stab_all, meta = _prep(x, inputs["edge_index"])
    wpack, vpack, gb, tb, cmean, crb, wg8 = _pack_weights(
        {k: np.asarray(v, dtype=np.float32)
         for k, v in inputs.items() if k != "edge_index"})
    meta["has_gb"] = bool(np.any(np.asarray(inputs["gcn_b"])))
    meta["has_tb"] = bool(np.any(np.asarray(inputs["topo_b"])))
    meta["has_lnb"] = bool(np.any(np.asarray(inputs["ln_node_b"]))
                           or np.any(np.asarray(inputs["ln_topo_b"])))
    meta["has_ab"] = bool(np.any(np.asarray(inputs["attn_b"])))

    nc = _build(meta)

    in_maps = []
    for c in range(NCORES):
        sl = slice(c * PC, (c + 1) * PC)
        in_maps.append({
            "xe": xe_all[c], "stab": stab_all[c],
            "xT": np.ascontiguousarray(xT[:, sl]),
            "invd": np.ascontiguousarray(invd[:, sl]),
            "wpack": wpack, "vpack": vpack, "gb": gb, "tbias": tb,
            "cmean": cmean, "crb": crb, "wg8": wg8,
        })
    global LAST_RESULTS, LAST_NC, LAST_INMAPS
    LAST_NC, LAST_INMAPS = nc, in_maps
    res = run_bass_kernel_spmd(nc, in_maps, core_ids=list(range(NCORES)))
    LAST_RESULTS = res
    out = np.concatenate(
        [np.asarray(res.results[c]["out"]).T for c in range(NCORES)], axis=0)
    return out[:N].astype(np.float32)
